# revision 2
# baseline (speedup 1.0000x reference)
"""3x3 zero-padded median filter (kornia MedianBlur semantics) on 8 trn2 cores.

Input  noised: (16, 3, 512, 512) f32, cover: same shape (pass-through).
Output (filtered, cover) — filtered is float32.

Sharding: pure data parallel over the 48 (B*C) images, 6 images per core.
Host packs each core's 6 images into one zero-separated stack I[3204, 514]
(one zero row between/around images gives the vertical zero padding; one
zero column each side gives the horizontal padding).  On device, partition p
owns R=25 consecutive output rows of the stack; vertical neighbors are
free-dim offsets (+-514), horizontal neighbors +-1.

median9 = med3( max3(column mins), med3(column mids), min3(column maxs) )
after sorting each vertical 3-column — an exact selection network (18
min/max tensor_tensor ops / pixel) on the vector engine.  The +1-column
shifted copies are produced on the (otherwise idle) scalar engine so every
DVE operand stays 4-byte aligned — which lets fp16 mode hit the DVE 2x
perf mode.

Internal dtype: float16 by default (~2x faster; output error ~= fp16
rounding of the exact median, rel err ~3e-4).  Set MEDIAN_FP32=1 in the
environment to compute bit-exactly in float32.
"""

import os

import numpy as np

import bass_rust
import concourse.bacc as bacc
import concourse.mybir as mybir
from concourse.tile import TileContext
from concourse.bass_utils import run_bass_kernel_spmd

B, CH, H, W = 16, 3, 512, 512
N_CORES = 8
IMGS = (B * CH) // N_CORES        # 6 images per core
SEP = H + 1                        # 513: image rows + 1 zero separator row
R = 25                             # output rows per partition (128*25 = 3200)
USE_FP32 = bool(int(os.environ.get("MEDIAN_FP32", "0")))
if USE_FP32:  # fp32 tiles are 2x bigger; smaller chunks to fit SBUF
    CHUNKS = [(0, 5), (5, 5), (10, 5), (15, 5), (20, 5)]
    LOADS = [(0, 7), (7, 5), (12, 5), (17, 5), (22, 5)]
else:         # (start row b, C rows) per chunk / input slot-row loads
    CHUNKS = [(0, 4), (4, 7), (11, 7), (18, 7)]
    LOADS = [(0, 6), (6, 7), (13, 7), (20, 7)]
WP = W + 2                         # 514: padded row width
IN_ROWS = 3204                     # >= 25*127 + 27, zero padded
OUT_ROWS = 128 * R                 # 3200

MN = mybir.AluOpType.min
MX = mybir.AluOpType.max

NP_DT = np.float32 if USE_FP32 else np.float16

_CACHE = {}


def _view(tile, r0, n, width, col0=0, rowstride=WP):
    """AP over `n` rows (stride `rowstride`) of `tile`, cols [col0, col0+width)."""
    ap = tile[:, r0 * rowstride + col0: r0 * rowstride + col0 + width].copy()
    ap.ap = bass_rust.VecI64Pair([list(ap.ap[0]), [rowstride, n], [1, width]])
    return ap


def _build():
    if "nc" in _CACHE:
        return _CACHE["nc"]
    dt = mybir.dt.float32 if USE_FP32 else mybir.dt.float16
    nc = bacc.Bacc(enable_partition_id=False)
    xin = nc.dram_tensor("xin", [IN_ROWS, WP], dt, kind="ExternalInput")
    yout = nc.dram_tensor("yout", [OUT_ROWS, WP], dt, kind="ExternalOutput")

    IN_FD = (R + 2) * WP          # 27 rows resident per partition
    WO = 512                      # output-frame row width

    with TileContext(nc) as tc:
        with tc.tile_pool(name="p", bufs=1) as pool, tc.tile_pool(name="io", bufs=2) as iop:
            tin = pool.tile([128, IN_FD], dt, tag="tin")
            for r0, n in LOADS:
                ap = xin[0:1, 0:1].copy()
                ap.ap = bass_rust.VecI64Pair([[R * WP, 128], [1, n * WP]])
                ap.offset = r0 * WP
                nc.sync.dma_start(tin[:, r0 * WP: (r0 + n) * WP], ap)

            for b, C in CHUNKS:
                # ---- vertical sort3, odd-slot shared pairs ----
                # pairs (in[s], in[s+1]) computed only at odd local slots s;
                # even output row r uses the pair at s=r+1 (elements b,c of
                # its window), odd row r the pair at s=r (elements a,b).
                np_ = (C + 1) // 2            # pairs == even-row count
                no = C // 2                   # odd-row count
                m_o = pool.tile([128, np_ * WP], dt, tag="m")
                M_o = pool.tile([128, np_ * WP], dt, tag="M")
                te = pool.tile([128, np_ * WP], dt, tag="te")

                def odd_slots(base, cnt):
                    return _view(tin, 0, cnt, WP, base * WP, 2 * WP)

                nc.vector.tensor_tensor(m_o[:], odd_slots(b + 1, np_), odd_slots(b + 2, np_), MN)
                nc.vector.tensor_tensor(M_o[:], odd_slots(b + 1, np_), odd_slots(b + 2, np_), MX)

                lo = pool.tile([128, C * WP], dt, tag="lo")
                hi = pool.tile([128, C * WP], dt, tag="hi")
                tv = pool.tile([128, C * WP], dt, tag="tv")

                def evens(t, cnt):       # rows 0,2,4,.. of a [C, WP] field
                    return _view(t, 0, cnt, WP, 0, 2 * WP)

                def odds(t, cnt):        # rows 1,3,5,..
                    return _view(t, 0, cnt, WP, WP, 2 * WP)

                def pair(t, cnt):        # first cnt pair rows (compact)
                    return _view(t, 0, cnt, WP, 0, WP)

                a_e = odd_slots(b, np_)          # in[b + 2k], k=0..ne-1
                nc.vector.tensor_tensor(evens(lo, np_), a_e, pair(m_o, np_), MN)
                nc.vector.tensor_tensor(evens(hi, np_), a_e, pair(M_o, np_), MX)
                nc.vector.tensor_tensor(pair(te, np_), a_e, pair(M_o, np_), MN)
                nc.vector.tensor_tensor(evens(tv, np_), pair(te, np_), pair(m_o, np_), MX)
                c_o = odd_slots(b + 3, no)       # in[b + 2k + 3]
                nc.vector.tensor_tensor(odds(lo, no), c_o, pair(m_o, no), MN)
                nc.vector.tensor_tensor(odds(hi, no), c_o, pair(M_o, no), MX)
                nc.vector.tensor_tensor(pair(te, no), c_o, pair(M_o, no), MN)
                nc.vector.tensor_tensor(odds(tv, no), pair(te, no), pair(m_o, no), MX)
                mid = tv

                # ---- +1 shifted copies ----
                # fp16: materialize on the scalar engine so every DVE operand
                # stays 4B-aligned (keeps the 2x perf mode).  fp32: the DVE
                # runs 1x regardless, so read the odd offset directly.
                if USE_FP32:
                    def S(src):
                        return _view(src, 0, C, WO, 1)
                else:
                    loS = pool.tile([128, C * WO], dt, tag="loS")
                    hiS = pool.tile([128, C * WO], dt, tag="hiS")
                    midS = pool.tile([128, C * WO], dt, tag="midS")
                    shifts = {id(lo): loS, id(hi): hiS, id(mid): midS}
                    for src, dstt in ((lo, loS), (hi, hiS), (mid, midS)):
                        nc.scalar.copy(_view(dstt, 0, C, WO, 0, WO), _view(src, 0, C, WO, 1))

                    def S(src):
                        return _view(shifts[id(src)], 0, C, WO, 0, WO)

                # ---- horizontal, output frame x' = window center x'+1 ----
                # field[x'] reads f[x'] (col0=0), fS[x'] (=f[x'+1]), f[x'+2]
                tA = pool.tile([128, C * WO], dt, tag="tA")   # pM / pmn / t1
                tB = pool.tile([128, C * WO], dt, tag="tB")   # pm / pmx / th / mm
                A = pool.tile([128, C * WO], dt, tag="A")     # maxlo / t2
                Bt = pool.tile([128, C * WO], dt, tag="B")    # minhi / t3
                out = iop.tile([128, C * WO], dt, tag="out")

                def V(t, col0=0, width=WO, stride=WO):
                    return _view(t, 0, C, width, col0, stride)

                # maxlo
                nc.vector.tensor_tensor(V(tA), V(lo, 0, WO, WP), S(lo), MX)
                nc.vector.tensor_tensor(V(A), V(tA), V(lo, 2, WO, WP), MX)
                # minhi
                nc.vector.tensor_tensor(V(tB), V(hi, 0, WO, WP), S(hi), MN)
                nc.vector.tensor_tensor(V(Bt), V(tB), V(hi, 2, WO, WP), MN)
                # medmid
                nc.vector.tensor_tensor(V(tA), V(mid, 0, WO, WP), S(mid), MN)   # pmn
                nc.vector.tensor_tensor(V(tB), V(mid, 0, WO, WP), S(mid), MX)   # pmx
                nc.vector.tensor_tensor(V(tB), V(tB), V(mid, 2, WO, WP), MN)     # th (in place)
                nc.vector.tensor_tensor(V(tB), V(tA), V(tB), MX)                 # mm (in place)
                # final med3(A, tB=medmid, Bt)
                nc.vector.tensor_tensor(V(tA), V(A), V(tB), MN)    # t1
                nc.vector.tensor_tensor(V(A), V(A), V(tB), MX)     # t2 (in place)
                nc.vector.tensor_tensor(V(Bt), V(A), V(Bt), MN)    # t3 (in place)
                nc.vector.tensor_tensor(V(out), V(tA), V(Bt), MX)

                # store: out row r -> yout row 25p + b + r, cols [1, 513)
                dst = yout[0:1, 0:1].copy()
                dst.ap = bass_rust.VecI64Pair([[R * WP, 128], [WP, C], [1, WO]])
                dst.offset = b * WP + 1
                nc.sync.dma_start(dst, V(out))

    nc.compile()
    _CACHE["nc"] = nc
    return nc


def _pack(core_imgs):
    """core_imgs: (IMGS, H, W) -> I[IN_ROWS, WP] in the device dtype."""
    I = np.zeros((IN_ROWS, WP), NP_DT)
    for i in range(IMGS):
        r0 = 1 + i * SEP
        I[r0: r0 + H, 1: 1 + W] = core_imgs[i].astype(NP_DT)
    return I


def _in_maps(noised):
    imgs = np.asarray(noised, dtype=np.float32).reshape(B * CH, H, W)
    return [{"xin": _pack(imgs[c * IMGS:(c + 1) * IMGS])} for c in range(N_CORES)]


def kernel(noised, cover):
    cover = np.asarray(cover)
    nc = _build()
    in_maps = _in_maps(noised)
    res = run_bass_kernel_spmd(nc, in_maps, core_ids=list(range(N_CORES)))
    out = np.empty((B * CH, H, W), np.float32)
    for c in range(N_CORES):
        Y = res.results[c]["yout"]
        for i in range(IMGS):
            out[c * IMGS + i] = Y[i * SEP: i * SEP + H, 1: 1 + W].astype(np.float32)
    filtered = out.reshape(B, CH, H, W)
    return filtered, cover



# revision 7
# speedup vs baseline: 1.0385x; 1.0385x over previous
"""3x3 zero-padded median filter (kornia MedianBlur semantics) on 8 trn2 cores.

Input  noised: (16, 3, 512, 512) f32, cover: same shape (pass-through).
Output (filtered, cover) — filtered is float32.

Sharding: pure data parallel over the 48 (B*C) images, 6 images per core.
Host packs each core's 6 images into one zero-separated stack I[3204, 514]
(one zero row between/around images gives the vertical zero padding; one
zero column each side gives the horizontal padding).  Partition p owns
R=25 consecutive output rows of the stack; the input window is 27 rows.

Algorithm (all DVE tensor_tensor min/max, fp16 2x mode, every operand
4-byte aligned):
  1. H-sort: per input row, sort each horizontal triple.  The three
     column taps come from three DMA loads of the same stack at column
     offsets 0/1/2 — no misaligned +1 reads, no shift copies.
       mH=min(t1,t2) MH=max(t1,t2); lo=min(t0,mH) hi=max(t0,MH)
       mid=max(min(t0,MH),mH)                       -> 6 ops/px
  2. V-merge: median9 = med3(max3(lo), med3(mid), min3(hi)) over the
     three vertical neighbors.  Vertical pairs live at row strides, so
     the half-rate shared-pair trick is alignment-free: pairs at odd
     slots s, even output r uses pair(r+1), odd r uses pair(r).
       pairs 4 ops at half rate + maxlo/minhi 1+1 + medmid 2 + final
       med3 4                                       -> 10 ops/px

Internal dtype float16 (exact median of fp16-rounded inputs; output
error ~= fp16 rounding, rel err ~2e-4).
"""

import numpy as np

import bass_rust
import concourse.bacc as bacc
import concourse.mybir as mybir
from concourse.tile import TileContext
from concourse.bass_utils import run_bass_kernel_spmd

B, CH, H, W = 16, 3, 512, 512
N_CORES = 8
IMGS = (B * CH) // N_CORES        # 6 images per core
SEP = H + 1                        # 513: image rows + 1 zero separator row
R = 25                             # output rows per partition (128*25 = 3200)
WP = W + 2                         # 514: padded input row width
WO = 512                           # output row width
IN_ROWS = 3204                     # 25*127 + 27 = 3202, zero padded
OUT_ROWS = 128 * R                 # 3200
INW = 27                           # input rows resident per partition

LOAD_CHUNKS = [(0, 6), (6, 6), (12, 6), (18, 6), (24, 3)]   # 27 rows
MERGE_CHUNKS = [(0, 8), (8, 8), (16, 8), (24, 1)]           # b even, 25 rows

MN = mybir.AluOpType.min
MX = mybir.AluOpType.max

NP_DT = np.float16

_CACHE = {}


def _view(tile, r0, n, width, col0=0, rowstride=WO):
    """AP over `n` rows (stride `rowstride`) of `tile`, cols [col0, col0+width)."""
    ap = tile[:, r0 * rowstride + col0: r0 * rowstride + col0 + width].copy()
    ap.ap = bass_rust.VecI64Pair([list(ap.ap[0]), [rowstride, n], [1, width]])
    return ap


def _build():
    if "nc" in _CACHE:
        return _CACHE["nc"]
    dt = mybir.dt.float16
    nc = bacc.Bacc(enable_partition_id=False)
    xin = nc.dram_tensor("xin", [IN_ROWS, WP], dt, kind="ExternalInput")
    yout = nc.dram_tensor("yout", [OUT_ROWS, WO], dt, kind="ExternalOutput")

    with TileContext(nc) as tc:
        with tc.tile_pool(name="p", bufs=1) as pool, \
             tc.tile_pool(name="ti", bufs=2) as tip, \
             tc.tile_pool(name="io", bufs=2) as iop:
            # full-height sorted-column planes
            L = pool.tile([128, INW * WO], dt, tag="L")
            Hh = pool.tile([128, INW * WO], dt, tag="H")
            M = pool.tile([128, INW * WO], dt, tag="M")

            def load(a, n):
                """three column taps of stack rows [a, a+n) -> tin0/1/2"""
                tins = []
                for k in range(3):
                    t = tip.tile([128, 6 * WO], dt, tag=f"tin{k}")
                    ap = xin[0:1, 0:1].copy()
                    ap.ap = bass_rust.VecI64Pair([[R * WP, 128], [WP, n], [1, WO]])
                    ap.offset = a * WP + k
                    nc.sync.dma_start(_view(t, 0, n, WO), ap)
                    tins.append(t)
                return tins

            def hsort(tins, a, n):
                t0, t1, t2 = (_view(t, 0, n, WO) for t in tins)
                mH = tip.tile([128, 6 * WO], dt, tag="mH")
                MH = tip.tile([128, 6 * WO], dt, tag="MH")
                m = _view(mH, 0, n, WO)
                Mv = _view(MH, 0, n, WO)
                nc.vector.tensor_tensor(m, t1, t2, MN)
                nc.vector.tensor_tensor(Mv, t1, t2, MX)
                nc.vector.tensor_tensor(_view(L, a, n, WO), t0, m, MN)
                nc.vector.tensor_tensor(_view(Hh, a, n, WO), t0, Mv, MX)
                te = _view(M, a, n, WO)
                nc.vector.tensor_tensor(te, t0, Mv, MN)
                nc.vector.tensor_tensor(te, te, m, MX)

            def merge(b, C):
                ne = (C + 1) // 2          # even outputs  r = b, b+2, ..
                no = C // 2                # odd outputs   r = b+1, b+3, ..
                npr = ne                   # pair slots s = b+1, b+3, ..
                # pairs tile also hosts t1 (rows 0..C) after pairs die
                prs = pool.tile([128, 4 * 4 * WO], dt, tag="prs")
                PL = lambda j0=0, n=npr: _view(prs, 0 * 4 + j0, n, WO)
                PH = lambda j0=0, n=npr: _view(prs, 1 * 4 + j0, n, WO)
                PN = lambda j0=0, n=npr: _view(prs, 2 * 4 + j0, n, WO)
                PX = lambda j0=0, n=npr: _view(prs, 3 * 4 + j0, n, WO)

                def odd(t, base, cnt):     # rows base, base+2, .. of plane t
                    return _view(t, 0, cnt, WO, base * WO, 2 * WO)

                nc.vector.tensor_tensor(PL(), odd(L, b + 1, npr), odd(L, b + 2, npr), MX)
                nc.vector.tensor_tensor(PH(), odd(Hh, b + 1, npr), odd(Hh, b + 2, npr), MN)
                nc.vector.tensor_tensor(PN(), odd(M, b + 1, npr), odd(M, b + 2, npr), MN)
                nc.vector.tensor_tensor(PX(), odd(M, b + 1, npr), odd(M, b + 2, npr), MX)

                ml = pool.tile([128, 8 * WO], dt, tag="ml")
                mh = pool.tile([128, 8 * WO], dt, tag="mh")
                md = pool.tile([128, 8 * WO], dt, tag="md")
                tS = pool.tile([128, 4 * WO], dt, tag="tS")
                out = iop.tile([128, 8 * WO], dt, tag="out")

                # even outputs: single = row r, pair slot s = r+1 (index j)
                nc.vector.tensor_tensor(odd(ml, 0, ne), odd(L, b, ne), PL(0, ne), MX)
                nc.vector.tensor_tensor(odd(mh, 0, ne), odd(Hh, b, ne), PH(0, ne), MN)
                nc.vector.tensor_tensor(_view(tS, 0, ne, WO), odd(M, b, ne), PX(0, ne), MN)
                nc.vector.tensor_tensor(odd(md, 0, ne), _view(tS, 0, ne, WO), PN(0, ne), MX)
                if no:
                    # odd outputs: single = row r+2, pair slot s = r (index j)
                    nc.vector.tensor_tensor(odd(ml, 1, no), PL(0, no), odd(L, b + 3, no), MX)
                    nc.vector.tensor_tensor(odd(mh, 1, no), PH(0, no), odd(Hh, b + 3, no), MN)
                    nc.vector.tensor_tensor(_view(tS, 0, no, WO), odd(M, b + 3, no), PX(0, no), MN)
                    nc.vector.tensor_tensor(odd(md, 1, no), _view(tS, 0, no, WO), PN(0, no), MX)

                # final med3(ml, md, mh); t1 reuses the pairs tile rows
                mlv = _view(ml, 0, C, WO)
                mdv = _view(md, 0, C, WO)
                mhv = _view(mh, 0, C, WO)
                t1 = _view(prs, 0, C, WO)
                nc.vector.tensor_tensor(t1, mlv, mdv, MN)
                nc.vector.tensor_tensor(mlv, mlv, mdv, MX)
                nc.vector.tensor_tensor(mlv, mlv, mhv, MN)
                nc.vector.tensor_tensor(_view(out, 0, C, WO), t1, mlv, MX)

                dst = yout[0:1, 0:1].copy()
                dst.ap = bass_rust.VecI64Pair([[R * WO, 128], [WO, C], [1, WO]])
                dst.offset = b * WO
                nc.sync.dma_start(dst, _view(out, 0, C, WO))

            # software pipeline: keep loads one chunk ahead of the sorts,
            # interleave merges as their source rows complete
            tins = {}
            tins[0] = load(*LOAD_CHUNKS[0])
            tins[1] = load(*LOAD_CHUNKS[1])
            hsort(tins.pop(0), *LOAD_CHUNKS[0])
            tins[2] = load(*LOAD_CHUNKS[2])
            hsort(tins.pop(1), *LOAD_CHUNKS[1])
            merge(*MERGE_CHUNKS[0])           # rows 0..9   (sorts 0-1)
            tins[3] = load(*LOAD_CHUNKS[3])
            hsort(tins.pop(2), *LOAD_CHUNKS[2])
            tins[4] = load(*LOAD_CHUNKS[4])
            hsort(tins.pop(3), *LOAD_CHUNKS[3])
            merge(*MERGE_CHUNKS[1])           # rows 8..17  (sorts 0-3)
            hsort(tins.pop(4), *LOAD_CHUNKS[4])
            merge(*MERGE_CHUNKS[2])           # rows 16..25
            merge(*MERGE_CHUNKS[3])           # row 24 (26)

    nc.compile()
    _CACHE["nc"] = nc
    return nc


def _pack(core_imgs):
    """core_imgs: (IMGS, H, W) -> I[IN_ROWS, WP] in the device dtype."""
    I = np.zeros((IN_ROWS, WP), NP_DT)
    for i in range(IMGS):
        r0 = 1 + i * SEP
        I[r0: r0 + H, 1: 1 + W] = core_imgs[i].astype(NP_DT)
    return I


def _in_maps(noised):
    imgs = np.asarray(noised, dtype=np.float32).reshape(B * CH, H, W)
    return [{"xin": _pack(imgs[c * IMGS:(c + 1) * IMGS])} for c in range(N_CORES)]


def kernel(noised, cover):
    cover = np.asarray(cover)
    nc = _build()
    in_maps = _in_maps(noised)
    res = run_bass_kernel_spmd(nc, in_maps, core_ids=list(range(N_CORES)))
    out = np.empty((B * CH, H, W), np.float32)
    for c in range(N_CORES):
        Y = res.results[c]["yout"]
        for i in range(IMGS):
            out[c * IMGS + i] = Y[i * SEP: i * SEP + H, :].astype(np.float32)
    filtered = out.reshape(B, CH, H, W)
    return filtered, cover
